# revision 32
# baseline (speedup 1.0000x reference)
"""Trainium2 Bass kernel for nn_CentroidDistance (Poincare centroid distance).

Math (reference):
    sq    = max(||x||^2 + ||c||^2 - 2 x.c, 0)
    denom = max((1-||x||^2)(1-||c||^2), 1e-12)
    arg   = 1 + 2 sq / denom
    d     = arccosh(max(arg, 1+eps))
    node_centroid_dist  = d * mask            # [1, N, C]
    graph_centroid_dist = sum(d*mask) / sum(mask)   # [1, C]

Strategy: data-parallel over the node dimension across 8 NeuronCores
(18750 nodes/core, padded to 18816 = 21 sweeps x 896). The host folds
the per-row factor a_i = mask_i/(1-sx_i) and per-column factor
b_j = 2/(1-sc_j) into an augmented bf16 GEMM so the single device GEMM
produces w_ij = mask_i * 2*sq_ij/denom_ij directly in PSUM:

    xhat (moving, [261, Npad] bf16), column i:
        rows 0..255 : a_i * x_i
        row  256    : a_i
        rows 257-260: hi(a_i*sx_i), lo(a_i*sx_i), hi(a_i*sx_i), lo(a_i*sx_i)
    cmov (stationary, [261, 256] bf16), column j:
        rows 0..255 : -2 * b_j * c_j
        row  256    : b_j * sc_j
        rows 257-260: hi(b_j), hi(b_j), lo(b_j), lo(b_j)

(hi/lo bf16 splits keep the large ||x||^2 * b_j term at ~fp32 accuracy.)
The constant cmov is the PE stationary so LDWEIGHTS churn stays low, and
the output lands transposed ([C, nodes]); the host transposes back
during unshard. Then arccosh(1+w) = ln(1 + w + sqrt((w+1)^2 - 1)):
    DVE: u = w + 1      (PSUM evac -> fp16, frees PSUM immediately)
    DVE: y = u * u      (fp16 tensor_tensor at 2x rate)
    ACT: s = Sqrt(y - 1)       (sqrt table set, phase-batched)
    DVE: z = u + s             ( = 1 + w + sqrt(w(w+2)) )
    ACT: d = Ln(z)             (ln table set, phase-batched)
Masked rows have w == 0 exactly -> d == 0 exactly. The Sqrt/Ln table-set
switches cost ~2.7us each, so ACT work is phase-batched per 7-sweep
group, with the next group's GEMM/evac stage software-pipelined across
the previous group's ACT phases to keep PE/DVE dense at boundaries.
graph_centroid_dist is reduced on host from the returned shards (the
device already folded the mask into the rows).
"""

import sys

for _p in ("/opt/trn_rl_repo",):
    if _p not in sys.path:
        sys.path.insert(0, _p)

import numpy as np
import ml_dtypes

import concourse.bass as bass
import concourse.tile as tile
from concourse import bacc, mybir
from concourse.bass_utils import run_bass_kernel_spmd


def _ensure_ntff_hook():
    """The agent image's `antenv` lacks `axon_hooks`; bass_utils hard-imports
    it for trace=True under axon. Shim the module and register the ctypes
    NTFF hook against the injected libaxon_pjrt.so."""
    import types
    try:
        import antenv.axon_hooks  # noqa: F401
        return
    except ImportError:
        pass
    import antenv
    mod = types.ModuleType("antenv.axon_hooks")
    mod._hook = None

    def set_axon_ntff_profile_hook(h):
        mod._hook = h

    def get_axon_ntff_profile_hook():
        return mod._hook

    mod.set_axon_ntff_profile_hook = set_axon_ntff_profile_hook
    mod.get_axon_ntff_profile_hook = get_axon_ntff_profile_hook
    sys.modules["antenv.axon_hooks"] = mod
    antenv.axon_hooks = mod

    so_path = "/opt/axon/libaxon_pjrt.so"
    try:
        from trn_agent_boot.trn_boot import _ntff_profile_via_ctypes
        hook = _ntff_profile_via_ctypes(so_path)
        if hook is not None:
            mod._hook = hook
    except Exception:
        pass


_ensure_ntff_hook()


BF16 = ml_dtypes.bfloat16
FP8 = ml_dtypes.float8_e4m3

N = 150000
D = 256
C = 256
N_CORES = 8
N_PER = N // N_CORES          # 18750 nodes per core
TILE = 128                    # nodes per matmul tile
SUP = 7                       # node-tiles per super-tile
SUP_NODES = SUP * TILE        # 896 nodes per super-tile
NSUP = 21                     # super-tiles per core
N_PAD = NSUP * SUP_NODES      # 18816 padded nodes per core
FD = SUP * C                  # 1792 free-dim elements per super-tile
K = 261                       # 256 + 1 + 4 augmented contraction dim
GROUP = 7                     # super-tiles per ACT table-set phase group

_PROGRAM_CACHE = {}


def build_program(out_dtype=mybir.dt.float32, y_on_dve=True):
    """Build the per-core Bass program (identical for all 8 cores).

    GEMM orientation: stationary = cmov halves (constant, so LDWEIGHTS
    amortizes), moving = xhat node columns.  PSUM gets w in [C, nodes]
    orientation; output DRAM is outT [C, N_PAD] and the host transposes
    during unshard.
    """
    from concourse.tile_rust import add_dep_helper

    nc = bacc.Bacc("TRN2", target_bir_lowering=False, debug=False,
                   enable_asserts=False)
    dt = mybir.dt

    xhat = nc.declare_dram_parameter("xhat", [K, N_PAD], dt.bfloat16,
                                     isOutput=False)
    cmov = nc.declare_dram_parameter("cmov", [K, C], dt.bfloat16,
                                     isOutput=False)
    outT = nc.declare_dram_parameter("outT", [C, N_PAD], out_dtype,
                                     isOutput=True)
    # tiny dummy output keeping the PE warm-up burst alive through DCE
    warm = nc.declare_dram_parameter("warm", [128, 1], dt.float32,
                                     isOutput=True)

    AF = mybir.ActivationFunctionType
    ALU = mybir.AluOpType

    # const AP for Sqrt's bias=-1.0 (only 0.0/1.0 pre-registered)
    _cm1 = nc.alloc_sbuf_tensor("const-f32-neg1", [128, 1], dt.float32)
    nc.gpsimd.memset(_cm1.ap(), -1.0)
    nc.const_aps.aps[(dt.float32, -1.0)] = _cm1.ap()
    nc.all_engine_barrier()

    KS = [(0, 128), (128, 128), (256, K - 256)]  # k-tile (start, size)
    SW = SUP_NODES            # 896 nodes per sweep
    NSW = NSUP                # 21 sweeps
    MM_SPLIT = ((0, 512), (512, 384))  # N<=512 fp32 psum-bank limit

    with tile.TileContext(nc) as tc:
        import contextlib
        ctx = contextlib.ExitStack()
        with ctx:
            cpool = ctx.enter_context(tc.tile_pool(name="cmov", bufs=1))
            xpool = ctx.enter_context(tc.tile_pool(name="xhat", bufs=8))
            psum_pool = ctx.enter_context(
                tc.tile_pool(name="psum", bufs=4, space="PSUM"))
            wpool = ctx.enter_context(
                tc.tile_pool(name="ws", bufs=GROUP + 3))
            rpool = ctx.enter_context(tc.tile_pool(name="r", bufs=GROUP + 2))
            spool = ctx.enter_context(tc.tile_pool(name="s", bufs=4))
            zpool = ctx.enter_context(
                tc.tile_pool(name="z", bufs=GROUP + 2))
            dpool = ctx.enter_context(tc.tile_pool(name="d", bufs=4))

            # load the replicated stationary operand once
            cm = []
            for (k0, ksz) in KS:
                t = cpool.tile([ksz, C], dt.bfloat16, tag=f"cm{k0}")
                nc.sync.dma_start(t[:], cmov[k0:k0 + ksz, :])
                cm.append(t)

            def load_xk(sw):
                n0 = sw * SW
                xk = []
                for (k0, ksz) in KS:
                    t = xpool.tile([ksz, SW], dt.bfloat16, tag=f"xk{k0}")
                    nc.sync.dma_start(t[:], xhat[k0:k0 + ksz, n0:n0 + SW])
                    xk.append(t)
                return xk

            # issue the first sweeps' loads before anything else so the
            # input pipeline is deep by the time real matmuls start
            xk_loaded = [load_xk(sw) for sw in range(GROUP)]

            # PE warm-up burst: dense matmuls get HAM to K=8/8 before
            # the steady-state loop (whose small gaps never re-warm it)
            pwarm = psum_pool.tile([TILE, 1024], dt.float32, tag="w")
            for i in range(40):
                nc.tensor.matmul(pwarm[:, 0:256], lhsT=cm[0][:, 0:128],
                                 rhs=cm[0][:, 0:C], start=True, stop=True)
            wtile = dpool.tile([TILE, 1], dt.float32, tag="warmout")
            nc.vector.tensor_scalar(wtile[:], pwarm[:, 0:1], 1.0, None,
                                    op0=ALU.mult)
            nc.sync.dma_start(warm[:, :], wtile[:])

            n_groups = NSW // GROUP
            last_d_inst = [None]
            last_s_inst = [None]
            first_flags = {}
            us = {}   # sw -> u tile
            ys = {}   # sw -> y tile
            zs = {}   # sw -> z tile

            def stage1(sw):
                xk = xk_loaded.pop(0) if xk_loaded else load_xk(sw)
                u = wpool.tile([TILE, 2 * SW], dt.float16, tag="u")
                for ch in range(2):
                    pw = psum_pool.tile([TILE, 1024], dt.float32, tag="w")
                    for ki in range(3):
                        for (f0, fsz) in MM_SPLIT:
                            nc.tensor.matmul(
                                pw[:, f0:f0 + fsz],
                                lhsT=cm[ki][:, ch * 128:(ch + 1) * 128],
                                rhs=xk[ki][:, f0:f0 + fsz],
                                start=(ki == 0), stop=(ki == 2),
                            )
                    nc.vector.tensor_scalar(
                        u[:, ch * SW:(ch + 1) * SW], pw[:, 0:SW],
                        1.0, None, op0=ALU.add)
                y = rpool.tile([TILE, 2 * SW], dt.float16, tag="y")
                nc.vector.tensor_mul(y[:], u[:], u[:])
                us[sw] = u
                ys[sw] = y

            def stage2(sw, first):
                s = spool.tile([TILE, 2 * SW], dt.float16, tag="s")
                s_inst = nc.scalar.activation(s[:], ys[sw][:], AF.Sqrt,
                                              bias=-1.0)
                if first and last_d_inst[0] is not None:
                    add_dep_helper(s_inst.ins, last_d_inst[0].ins,
                                   sync=False, reason="ACT phase order")
                last_s_inst[0] = s_inst
                z = zpool.tile([TILE, 2 * SW], dt.float16, tag="z")
                nc.vector.tensor_add(z[:], us[sw][:], s[:])
                zs[sw] = z
                del us[sw], ys[sw]

            def stage3(sw, first):
                n0 = sw * SW
                d_t = dpool.tile([TILE, 2 * SW], out_dtype, tag="d")
                d_inst = nc.scalar.activation(d_t[:], zs[sw][:], AF.Ln)
                if first:
                    add_dep_helper(d_inst.ins, last_s_inst[0].ins,
                                   sync=False, reason="ACT phase order")
                last_d_inst[0] = d_inst
                for ch in range(2):
                    nc.sync.dma_start(
                        outT[ch * 128:(ch + 1) * 128, n0:n0 + SW],
                        d_t[:, ch * SW:(ch + 1) * SW])
                del zs[sw]

            # Pipeline schedule (ACT table-phase blocks stay contiguous):
            #   fill : stage1(g0) 1:1 with s/z(g0)
            #   steady, per g: [d(g-1) x G | s/z(g) x G] interleaved 1:2
            #                  with stage1(g) units
            #   tail : d(last) x G
            for si in range(GROUP):
                stage1(si)
                if si >= 1:
                    stage2(si - 1, first=(si == 1))
            stage2(GROUP - 1, first=False)

            for g in range(1, n_groups):
                for k in range(2 * GROUP):
                    if k % 2 == 0:
                        stage1(g * GROUP + k // 2)
                    if k < GROUP:
                        stage3((g - 1) * GROUP + k, first=(k == 0))
                    else:
                        kk = k - GROUP
                        stage2(g * GROUP + kk, first=(kk == 0))
            for k in range(GROUP):
                stage3((n_groups - 1) * GROUP + k, first=(k == 0))

    nc.compile()
    return nc


def get_program(**kw):
    key = tuple(sorted(kw.items()))
    if key not in _PROGRAM_CACHE:
        _PROGRAM_CACHE[key] = build_program(**kw)
    return _PROGRAM_CACHE[key]


Y_ON_DVE = True


def _hi_lo(v):
    hi = v.astype(BF16)
    lo = (v - hi.astype(np.float32)).astype(BF16)
    return hi, lo


def host_prep(node_repr, mask, centroid_weight):
    """Build per-core xhat shards and the replicated cmov matrix."""
    x = np.ascontiguousarray(node_repr, dtype=np.float32)
    m = np.ascontiguousarray(mask, dtype=np.float32).reshape(-1)
    c = np.ascontiguousarray(centroid_weight, dtype=np.float32)

    sx = np.einsum("nd,nd->n", x, x, dtype=np.float32)
    sc = np.einsum("cd,cd->c", c, c, dtype=np.float32)
    a = m / (1.0 - sx)                      # mask folded in
    b = 2.0 / (1.0 - sc)

    # moving operand [K, C]
    cmov = np.zeros((K, C), dtype=BF16)
    cmov[0:D, :] = (-2.0 * b[:, None] * c).T.astype(BF16)
    cmov[D, :] = (b * sc).astype(BF16)
    bhi, blo = _hi_lo(b)
    cmov[D + 1, :] = bhi
    cmov[D + 2, :] = bhi
    cmov[D + 3, :] = blo
    cmov[D + 4, :] = blo

    v = a * sx
    vhi, vlo = _hi_lo(v)
    ax = (x * a[:, None]).astype(BF16)      # [N, D]
    abf = a.astype(BF16)

    xhats = []
    for i in range(N_CORES):
        n0, n1 = i * N_PER, (i + 1) * N_PER
        xh = np.zeros((K, N_PAD), dtype=BF16)
        xh[0:D, :N_PER] = ax[n0:n1].T
        xh[D, :N_PER] = abf[n0:n1]
        xh[D + 1, :N_PER] = vhi[n0:n1]
        xh[D + 2, :N_PER] = vlo[n0:n1]
        xh[D + 3, :N_PER] = vhi[n0:n1]
        xh[D + 4, :N_PER] = vlo[n0:n1]
        xhats.append(xh)
    return xhats, cmov, m


def kernel(node_repr, mask, centroid_weight, trace=False, out_dtype="float16"):
    xhats, cmov, m = host_prep(node_repr, mask, centroid_weight)

    odt = mybir.dt.float32 if out_dtype == "float32" else mybir.dt.float16
    nc = get_program(out_dtype=odt)

    in_maps = [{"xhat": xhats[i], "cmov": cmov} for i in range(N_CORES)]
    res = run_bass_kernel_spmd(nc, in_maps, core_ids=list(range(N_CORES)),
                               trace=trace)

    parts = []
    gsum = np.zeros((C,), dtype=np.float32)
    for i in range(N_CORES):
        o = np.asarray(res.results[i]["outT"][:, :N_PER], dtype=np.float32)
        gsum += o.sum(axis=1, dtype=np.float32)
        parts.append(o.T)

    node_centroid_dist = np.ascontiguousarray(
        np.concatenate(parts, axis=0))[None]  # [1, N, C]
    msum = m.sum(dtype=np.float32)
    graph_centroid_dist = (gsum / msum)[None]
    if trace:
        kernel.last_result = res
    return graph_centroid_dist, node_centroid_dist


# revision 33
# speedup vs baseline: 1.0624x; 1.0624x over previous
"""Trainium2 Bass kernel for nn_CentroidDistance (Poincare centroid distance).

Math (reference):
    sq    = max(||x||^2 + ||c||^2 - 2 x.c, 0)
    denom = max((1-||x||^2)(1-||c||^2), 1e-12)
    arg   = 1 + 2 sq / denom
    d     = arccosh(max(arg, 1+eps))
    node_centroid_dist  = d * mask            # [1, N, C]
    graph_centroid_dist = sum(d*mask) / sum(mask)   # [1, C]

Strategy: data-parallel over the node dimension across 8 NeuronCores
(18750 nodes/core, padded to 18816 = 21 sweeps x 896). The host folds
the per-row factor a_i = mask_i/(1-sx_i) and per-column factor
b_j = 2/(1-sc_j) into an augmented bf16 GEMM so the single device GEMM
produces w_ij = mask_i * 2*sq_ij/denom_ij directly in PSUM:

    xhat (moving, [261, Npad] bf16), column i:
        rows 0..255 : a_i * x_i
        row  256    : a_i
        rows 257-260: hi(a_i*sx_i), lo(a_i*sx_i), hi(a_i*sx_i), lo(a_i*sx_i)
    cmov (stationary, [261, 256] bf16), column j:
        rows 0..255 : -2 * b_j * c_j
        row  256    : b_j * sc_j
        rows 257-260: hi(b_j), hi(b_j), lo(b_j), lo(b_j)

(hi/lo bf16 splits keep the large ||x||^2 * b_j term at ~fp32 accuracy.)
The constant cmov is the PE stationary so LDWEIGHTS churn stays low, and
the output lands transposed ([C, nodes]); the host transposes back
during unshard. Then arccosh(1+w) = ln(1 + w + sqrt((w+1)^2 - 1)):
    DVE: u = w + 1      (PSUM evac -> fp16, frees PSUM immediately)
    DVE: y = u * u      (fp16 tensor_tensor at 2x rate)
    ACT: s = Sqrt(y - 1)       (sqrt table set, phase-batched)
    DVE: z = u + s             ( = 1 + w + sqrt(w(w+2)) )
    ACT: d = Ln(z)             (ln table set, phase-batched)
Masked rows have w == 0 exactly -> d == 0 exactly. The Sqrt/Ln table-set
switches cost ~2.7us each, so ACT work is phase-batched per 7-sweep
group, with the next group's GEMM/evac stage software-pipelined across
the previous group's ACT phases to keep PE/DVE dense at boundaries.
graph_centroid_dist is reduced on host from the returned shards (the
device already folded the mask into the rows).
"""

import sys

for _p in ("/opt/trn_rl_repo",):
    if _p not in sys.path:
        sys.path.insert(0, _p)

import numpy as np
import ml_dtypes

import concourse.bass as bass
import concourse.tile as tile
from concourse import bacc, mybir
from concourse.bass_utils import run_bass_kernel_spmd


def _ensure_ntff_hook():
    """The agent image's `antenv` lacks `axon_hooks`; bass_utils hard-imports
    it for trace=True under axon. Shim the module and register the ctypes
    NTFF hook against the injected libaxon_pjrt.so."""
    import types
    try:
        import antenv.axon_hooks  # noqa: F401
        return
    except ImportError:
        pass
    import antenv
    mod = types.ModuleType("antenv.axon_hooks")
    mod._hook = None

    def set_axon_ntff_profile_hook(h):
        mod._hook = h

    def get_axon_ntff_profile_hook():
        return mod._hook

    mod.set_axon_ntff_profile_hook = set_axon_ntff_profile_hook
    mod.get_axon_ntff_profile_hook = get_axon_ntff_profile_hook
    sys.modules["antenv.axon_hooks"] = mod
    antenv.axon_hooks = mod

    so_path = "/opt/axon/libaxon_pjrt.so"
    try:
        from trn_agent_boot.trn_boot import _ntff_profile_via_ctypes
        hook = _ntff_profile_via_ctypes(so_path)
        if hook is not None:
            mod._hook = hook
    except Exception:
        pass


_ensure_ntff_hook()


BF16 = ml_dtypes.bfloat16
FP8 = ml_dtypes.float8_e4m3

N = 150000
D = 256
C = 256
N_CORES = 8
N_PER = N // N_CORES          # 18750 nodes per core
TILE = 128                    # nodes per matmul tile
SUP = 7                       # node-tiles per super-tile
SUP_NODES = SUP * TILE        # 896 nodes per super-tile
NSUP = 21                     # super-tiles per core
N_PAD = NSUP * SUP_NODES      # 18816 padded nodes per core
FD = SUP * C                  # 1792 free-dim elements per super-tile
K = 261                       # 256 + 1 + 4 augmented contraction dim
GROUP = 7                     # super-tiles per ACT table-set phase group

_PROGRAM_CACHE = {}


def build_program(out_dtype=mybir.dt.float32, y_on_dve=True):
    """Build the per-core Bass program (identical for all 8 cores).

    GEMM orientation: stationary = cmov halves (constant, so LDWEIGHTS
    amortizes), moving = xhat node columns.  PSUM gets w in [C, nodes]
    orientation; output DRAM is outT [C, N_PAD] and the host transposes
    during unshard.
    """
    from concourse.tile_rust import add_dep_helper

    nc = bacc.Bacc("TRN2", target_bir_lowering=False, debug=False,
                   enable_asserts=False)
    dt = mybir.dt

    xhat = nc.declare_dram_parameter("xhat", [K, N_PAD], dt.bfloat16,
                                     isOutput=False)
    cmov = nc.declare_dram_parameter("cmov", [K, C], dt.bfloat16,
                                     isOutput=False)
    outT = nc.declare_dram_parameter("outT", [C, N_PAD], out_dtype,
                                     isOutput=True)
    # tiny dummy output keeping the PE warm-up burst alive through DCE
    warm = nc.declare_dram_parameter("warm", [128, 1], dt.float32,
                                     isOutput=True)

    AF = mybir.ActivationFunctionType
    ALU = mybir.AluOpType

    # const AP for Sqrt's bias=-1.0 (only 0.0/1.0 pre-registered)
    _cm1 = nc.alloc_sbuf_tensor("const-f32-neg1", [128, 1], dt.float32)
    nc.gpsimd.memset(_cm1.ap(), -1.0)
    nc.const_aps.aps[(dt.float32, -1.0)] = _cm1.ap()
    nc.all_engine_barrier()

    KS = [(0, 128), (128, 128), (256, K - 256)]  # k-tile (start, size)
    SW = SUP_NODES            # 896 nodes per sweep
    NSW = NSUP                # 21 sweeps
    MM_SPLIT = ((0, 512), (512, 384))  # N<=512 fp32 psum-bank limit

    with tile.TileContext(nc) as tc:
        import contextlib
        ctx = contextlib.ExitStack()
        with ctx:
            cpool = ctx.enter_context(tc.tile_pool(name="cmov", bufs=1))
            xpool = ctx.enter_context(tc.tile_pool(name="xhat", bufs=8))
            psum_pool = ctx.enter_context(
                tc.tile_pool(name="psum", bufs=4, space="PSUM"))
            wpool = ctx.enter_context(
                tc.tile_pool(name="ws", bufs=GROUP + 3))
            rpool = ctx.enter_context(tc.tile_pool(name="r", bufs=GROUP + 2))
            spool = ctx.enter_context(tc.tile_pool(name="s", bufs=4))
            zpool = ctx.enter_context(
                tc.tile_pool(name="z", bufs=GROUP + 2))
            dpool = ctx.enter_context(tc.tile_pool(name="d", bufs=4))

            # load the replicated stationary operand once
            cm = []
            for (k0, ksz) in KS:
                t = cpool.tile([ksz, C], dt.bfloat16, tag=f"cm{k0}")
                nc.sync.dma_start(t[:], cmov[k0:k0 + ksz, :])
                cm.append(t)

            def load_xk(sw):
                n0 = sw * SW
                xk = []
                for (k0, ksz) in KS:
                    t = xpool.tile([ksz, SW], dt.bfloat16, tag=f"xk{k0}")
                    nc.sync.dma_start(t[:], xhat[k0:k0 + ksz, n0:n0 + SW])
                    xk.append(t)
                return xk

            # issue the first sweeps' loads before anything else so the
            # input pipeline is deep by the time real matmuls start
            xk_loaded = [load_xk(sw) for sw in range(GROUP)]

            # PE warm-up burst: dense matmuls get HAM to K=8/8 before
            # the steady-state loop (whose small gaps never re-warm it)
            pwarm = psum_pool.tile([TILE, 1024], dt.float32, tag="w")
            for i in range(40):
                nc.tensor.matmul(pwarm[:, 0:256], lhsT=cm[0][:, 0:128],
                                 rhs=cm[0][:, 0:C], start=True, stop=True)
            wtile = dpool.tile([TILE, 1], dt.float32, tag="warmout")
            nc.vector.tensor_scalar(wtile[:], pwarm[:, 0:1], 1.0, None,
                                    op0=ALU.mult)
            nc.sync.dma_start(warm[:, :], wtile[:])

            n_groups = NSW // GROUP
            last_d_inst = [None]
            last_s_inst = [None]
            first_flags = {}
            us = {}   # sw -> u tile
            ys = {}   # sw -> y tile
            zs = {}   # sw -> z tile

            def stage1(sw):
                xk = xk_loaded.pop(0) if xk_loaded else load_xk(sw)
                u = wpool.tile([TILE, 2 * SW], dt.float16, tag="u")
                for ch in range(2):
                    pw = psum_pool.tile([TILE, 1024], dt.float32, tag="w")
                    for ki in range(3):
                        for (f0, fsz) in MM_SPLIT:
                            nc.tensor.matmul(
                                pw[:, f0:f0 + fsz],
                                lhsT=cm[ki][:, ch * 128:(ch + 1) * 128],
                                rhs=xk[ki][:, f0:f0 + fsz],
                                start=(ki == 0), stop=(ki == 2),
                            )
                    nc.vector.tensor_scalar(
                        u[:, ch * SW:(ch + 1) * SW], pw[:, 0:SW],
                        1.0, None, op0=ALU.add)
                y = rpool.tile([TILE, 2 * SW], dt.float16, tag="y")
                nc.vector.tensor_mul(y[:], u[:], u[:])
                us[sw] = u
                ys[sw] = y

            def stage2(sw, first):
                s = spool.tile([TILE, 2 * SW], dt.float16, tag="s")
                s_inst = nc.scalar.activation(s[:], ys[sw][:], AF.Sqrt,
                                              bias=-1.0)
                if first and last_d_inst[0] is not None:
                    add_dep_helper(s_inst.ins, last_d_inst[0].ins,
                                   sync=False, reason="ACT phase order")
                last_s_inst[0] = s_inst
                z = zpool.tile([TILE, 2 * SW], dt.float16, tag="z")
                nc.vector.tensor_add(z[:], us[sw][:], s[:])
                zs[sw] = z
                del us[sw], ys[sw]

            def stage3(sw, first):
                n0 = sw * SW
                d_t = dpool.tile([TILE, 2 * SW], out_dtype, tag="d")
                d_inst = nc.scalar.activation(d_t[:], zs[sw][:], AF.Ln)
                if first:
                    add_dep_helper(d_inst.ins, last_s_inst[0].ins,
                                   sync=False, reason="ACT phase order")
                last_d_inst[0] = d_inst
                for ch in range(2):
                    nc.sync.dma_start(
                        outT[ch * 128:(ch + 1) * 128, n0:n0 + SW],
                        d_t[:, ch * SW:(ch + 1) * SW])
                del zs[sw]

            # group 0 stage1 runs standalone (PE warm-up covers the ramp)
            for si in range(GROUP):
                stage1(si)
            # groups g: stage1(g) interleaved across group (g-1)'s phases
            for g in range(1, n_groups + 1):
                prev = (g - 1) * GROUP
                for k in range(2 * GROUP):
                    if k % 2 == 0 and g < n_groups:
                        sw = g * GROUP + k // 2
                        if sw < NSW:
                            stage1(sw)
                    if k < GROUP:
                        stage2(prev + k, first=(k == 0))
                    else:
                        stage3(prev + (k - GROUP), first=(k == GROUP))

    nc.compile()
    return nc


def get_program(**kw):
    key = tuple(sorted(kw.items()))
    if key not in _PROGRAM_CACHE:
        _PROGRAM_CACHE[key] = build_program(**kw)
    return _PROGRAM_CACHE[key]


Y_ON_DVE = True


def _hi_lo(v):
    hi = v.astype(BF16)
    lo = (v - hi.astype(np.float32)).astype(BF16)
    return hi, lo


def host_prep(node_repr, mask, centroid_weight):
    """Build per-core xhat shards and the replicated cmov matrix."""
    x = np.ascontiguousarray(node_repr, dtype=np.float32)
    m = np.ascontiguousarray(mask, dtype=np.float32).reshape(-1)
    c = np.ascontiguousarray(centroid_weight, dtype=np.float32)

    sx = np.einsum("nd,nd->n", x, x, dtype=np.float32)
    sc = np.einsum("cd,cd->c", c, c, dtype=np.float32)
    a = m / (1.0 - sx)                      # mask folded in
    b = 2.0 / (1.0 - sc)

    # moving operand [K, C]
    cmov = np.zeros((K, C), dtype=BF16)
    cmov[0:D, :] = (-2.0 * b[:, None] * c).T.astype(BF16)
    cmov[D, :] = (b * sc).astype(BF16)
    bhi, blo = _hi_lo(b)
    cmov[D + 1, :] = bhi
    cmov[D + 2, :] = bhi
    cmov[D + 3, :] = blo
    cmov[D + 4, :] = blo

    v = a * sx
    vhi, vlo = _hi_lo(v)
    ax = (x * a[:, None]).astype(BF16)      # [N, D]
    abf = a.astype(BF16)

    xhats = []
    for i in range(N_CORES):
        n0, n1 = i * N_PER, (i + 1) * N_PER
        xh = np.zeros((K, N_PAD), dtype=BF16)
        xh[0:D, :N_PER] = ax[n0:n1].T
        xh[D, :N_PER] = abf[n0:n1]
        xh[D + 1, :N_PER] = vhi[n0:n1]
        xh[D + 2, :N_PER] = vlo[n0:n1]
        xh[D + 3, :N_PER] = vhi[n0:n1]
        xh[D + 4, :N_PER] = vlo[n0:n1]
        xhats.append(xh)
    return xhats, cmov, m


def kernel(node_repr, mask, centroid_weight, trace=False, out_dtype="float16"):
    xhats, cmov, m = host_prep(node_repr, mask, centroid_weight)

    odt = mybir.dt.float32 if out_dtype == "float32" else mybir.dt.float16
    nc = get_program(out_dtype=odt)

    in_maps = [{"xhat": xhats[i], "cmov": cmov} for i in range(N_CORES)]
    res = run_bass_kernel_spmd(nc, in_maps, core_ids=list(range(N_CORES)),
                               trace=trace)

    parts = []
    gsum = np.zeros((C,), dtype=np.float32)
    for i in range(N_CORES):
        o = np.asarray(res.results[i]["outT"][:, :N_PER], dtype=np.float32)
        gsum += o.sum(axis=1, dtype=np.float32)
        parts.append(o.T)

    node_centroid_dist = np.ascontiguousarray(
        np.concatenate(parts, axis=0))[None]  # [1, N, C]
    msum = m.sum(dtype=np.float32)
    graph_centroid_dist = (gsum / msum)[None]
    if trace:
        kernel.last_result = res
    return graph_centroid_dist, node_centroid_dist


# revision 36
# speedup vs baseline: 1.0760x; 1.0128x over previous
"""Trainium2 Bass kernel for nn_CentroidDistance (Poincare centroid distance).

Math (reference):
    sq    = max(||x||^2 + ||c||^2 - 2 x.c, 0)
    denom = max((1-||x||^2)(1-||c||^2), 1e-12)
    arg   = 1 + 2 sq / denom
    d     = arccosh(max(arg, 1+eps))
    node_centroid_dist  = d * mask            # [1, N, C]
    graph_centroid_dist = sum(d*mask) / sum(mask)   # [1, C]

Strategy: data-parallel over the node dimension across 8 NeuronCores
(18750 nodes/core, padded to 18816 = 21 sweeps x 896). The host folds
the per-row factor a_i = mask_i/(1-sx_i) and per-column factor
b_j = 2/(1-sc_j) into an augmented bf16 GEMM so the single device GEMM
produces w_ij = mask_i * 2*sq_ij/denom_ij directly in PSUM:

    xhat (moving, [261, Npad] bf16), column i:
        rows 0..255 : a_i * x_i
        row  256    : a_i
        rows 257-260: hi(a_i*sx_i), lo(a_i*sx_i), hi(a_i*sx_i), lo(a_i*sx_i)
    cmov (stationary, [261, 256] bf16), column j:
        rows 0..255 : -2 * b_j * c_j
        row  256    : b_j * sc_j
        rows 257-260: hi(b_j), hi(b_j), lo(b_j), lo(b_j)

(hi/lo bf16 splits keep the large ||x||^2 * b_j term at ~fp32 accuracy.)
The constant cmov is the PE stationary so LDWEIGHTS churn stays low, and
the output lands transposed ([C, nodes]); the host transposes back
during unshard. Then arccosh(1+w) = ln(1 + w + sqrt((w+1)^2 - 1)):
    DVE: u = w + 1      (PSUM evac -> fp16, frees PSUM immediately)
    DVE: y = u * u      (fp16 tensor_tensor at 2x rate)
    ACT: s = Sqrt(y - 1)       (sqrt table set, phase-batched)
    DVE: z = u + s             ( = 1 + w + sqrt(w(w+2)) )
    ACT: d = Ln(z)             (ln table set, phase-batched)
Masked rows have w == 0 exactly -> d == 0 exactly. The Sqrt/Ln table-set
switches cost ~2.7us each, so ACT work is phase-batched per 7-sweep
group, with the next group's GEMM/evac stage software-pipelined across
the previous group's ACT phases to keep PE/DVE dense at boundaries.
graph_centroid_dist is reduced on host from the returned shards (the
device already folded the mask into the rows).
"""

import sys

for _p in ("/opt/trn_rl_repo",):
    if _p not in sys.path:
        sys.path.insert(0, _p)

import numpy as np
import ml_dtypes

import concourse.bass as bass
import concourse.tile as tile
from concourse import bacc, mybir
from concourse.bass_utils import run_bass_kernel_spmd


def _ensure_ntff_hook():
    """The agent image's `antenv` lacks `axon_hooks`; bass_utils hard-imports
    it for trace=True under axon. Shim the module and register the ctypes
    NTFF hook against the injected libaxon_pjrt.so."""
    import types
    try:
        import antenv.axon_hooks  # noqa: F401
        return
    except ImportError:
        pass
    import antenv
    mod = types.ModuleType("antenv.axon_hooks")
    mod._hook = None

    def set_axon_ntff_profile_hook(h):
        mod._hook = h

    def get_axon_ntff_profile_hook():
        return mod._hook

    mod.set_axon_ntff_profile_hook = set_axon_ntff_profile_hook
    mod.get_axon_ntff_profile_hook = get_axon_ntff_profile_hook
    sys.modules["antenv.axon_hooks"] = mod
    antenv.axon_hooks = mod

    so_path = "/opt/axon/libaxon_pjrt.so"
    try:
        from trn_agent_boot.trn_boot import _ntff_profile_via_ctypes
        hook = _ntff_profile_via_ctypes(so_path)
        if hook is not None:
            mod._hook = hook
    except Exception:
        pass


_ensure_ntff_hook()


BF16 = ml_dtypes.bfloat16
FP8 = ml_dtypes.float8_e4m3

N = 150000
D = 256
C = 256
N_CORES = 8
N_PER = N // N_CORES          # 18750 nodes per core
TILE = 128                    # nodes per matmul tile
SUP = 7                       # node-tiles per super-tile
SUP_NODES = SUP * TILE        # 896 nodes per super-tile
NSUP = 21                     # super-tiles per core
N_PAD = NSUP * SUP_NODES      # 18816 padded nodes per core
FD = SUP * C                  # 1792 free-dim elements per super-tile
K = 261                       # 256 + 1 + 4 augmented contraction dim
GROUP = 7                     # super-tiles per ACT table-set phase group

_PROGRAM_CACHE = {}


def build_program(out_dtype=mybir.dt.float32, y_on_dve=True):
    """Build the per-core Bass program (identical for all 8 cores).

    GEMM orientation: stationary = cmov halves (constant, so LDWEIGHTS
    amortizes), moving = xhat node columns.  PSUM gets w in [C, nodes]
    orientation; output DRAM is outT [C, N_PAD] and the host transposes
    during unshard.
    """
    from concourse.tile_rust import add_dep_helper

    nc = bacc.Bacc("TRN2", target_bir_lowering=False, debug=False,
                   enable_asserts=False)
    dt = mybir.dt

    xhat = nc.declare_dram_parameter("xhat", [K, N_PAD], dt.bfloat16,
                                     isOutput=False)
    cmov = nc.declare_dram_parameter("cmov", [K, C], dt.bfloat16,
                                     isOutput=False)
    outT = nc.declare_dram_parameter("outT", [C, N_PAD], out_dtype,
                                     isOutput=True)
    # tiny dummy output keeping the PE warm-up burst alive through DCE
    warm = nc.declare_dram_parameter("warm", [128, 1], dt.float32,
                                     isOutput=True)

    AF = mybir.ActivationFunctionType
    ALU = mybir.AluOpType

    # const AP for Sqrt's bias=-1.0 (only 0.0/1.0 pre-registered)
    _cm1 = nc.alloc_sbuf_tensor("const-f32-neg1", [128, 1], dt.float32)
    nc.gpsimd.memset(_cm1.ap(), -1.0)
    nc.const_aps.aps[(dt.float32, -1.0)] = _cm1.ap()
    nc.all_engine_barrier()

    KS = [(0, 128), (128, 128), (256, K - 256)]  # k-tile (start, size)
    SW = SUP_NODES            # 896 nodes per sweep
    NSW = NSUP                # 21 sweeps
    MM_SPLIT = ((0, 512), (512, 384))  # N<=512 fp32 psum-bank limit

    with tile.TileContext(nc) as tc:
        import contextlib
        ctx = contextlib.ExitStack()
        with ctx:
            cpool = ctx.enter_context(tc.tile_pool(name="cmov", bufs=1))
            xpool = ctx.enter_context(tc.tile_pool(name="xhat", bufs=8))
            psum_pool = ctx.enter_context(
                tc.tile_pool(name="psum", bufs=4, space="PSUM"))
            wpool = ctx.enter_context(
                tc.tile_pool(name="ws", bufs=GROUP + 3))
            rpool = ctx.enter_context(tc.tile_pool(name="r", bufs=GROUP + 2))
            spool = ctx.enter_context(tc.tile_pool(name="s", bufs=4))
            zpool = ctx.enter_context(
                tc.tile_pool(name="z", bufs=GROUP + 2))
            dpool = ctx.enter_context(tc.tile_pool(name="d", bufs=4))

            # load the replicated stationary operand once
            cm = []
            for (k0, ksz) in KS:
                t = cpool.tile([ksz, C], dt.bfloat16, tag=f"cm{k0}")
                nc.sync.dma_start(t[:], cmov[k0:k0 + ksz, :])
                cm.append(t)

            def load_xk(sw):
                n0 = sw * SW
                xk = []
                for (k0, ksz) in KS:
                    t = xpool.tile([ksz, SW], dt.bfloat16, tag=f"xk{k0}")
                    nc.sync.dma_start(t[:], xhat[k0:k0 + ksz, n0:n0 + SW])
                    xk.append(t)
                return xk

            # issue the first sweeps' loads before anything else so the
            # input pipeline is deep by the time real matmuls start
            xk_loaded = [load_xk(sw) for sw in range(GROUP)]

            # PE warm-up burst: dense matmuls get HAM to K=8/8 before
            # the steady-state loop (whose small gaps never re-warm it)
            pwarm = psum_pool.tile([TILE, 1024], dt.float32, tag="w")
            for i in range(40):
                nc.tensor.matmul(pwarm[:, 0:256], lhsT=cm[0][:, 0:128],
                                 rhs=cm[0][:, 0:C], start=True, stop=True)
            wtile = dpool.tile([TILE, 1], dt.float32, tag="warmout")
            nc.vector.tensor_scalar(wtile[:], pwarm[:, 0:1], 1.0, None,
                                    op0=ALU.mult)
            nc.sync.dma_start(warm[:, :], wtile[:])

            n_groups = NSW // GROUP
            last_d_inst = [None]
            last_s_inst = [None]
            first_flags = {}
            us = {}   # sw -> u tile
            ys = {}   # sw -> y tile
            zs = {}   # sw -> z tile

            def stage1(sw):
                xk = xk_loaded.pop(0) if xk_loaded else load_xk(sw)
                u = wpool.tile([TILE, 2 * SW], dt.float16, tag="u")
                for ch in range(2):
                    pw = psum_pool.tile([TILE, 1024], dt.float32, tag="w")
                    for ki in range(3):
                        for (f0, fsz) in MM_SPLIT:
                            nc.tensor.matmul(
                                pw[:, f0:f0 + fsz],
                                lhsT=cm[ki][:, ch * 128:(ch + 1) * 128],
                                rhs=xk[ki][:, f0:f0 + fsz],
                                start=(ki == 0), stop=(ki == 2),
                            )
                    nc.vector.tensor_scalar(
                        u[:, ch * SW:(ch + 1) * SW], pw[:, 0:SW],
                        1.0, None, op0=ALU.add)
                y = rpool.tile([TILE, 2 * SW], dt.float16, tag="y")
                nc.vector.tensor_mul(y[:], u[:], u[:])
                us[sw] = u
                ys[sw] = y

            def stage2(sw, first):
                s = spool.tile([TILE, 2 * SW], dt.float16, tag="s")
                s_inst = nc.scalar.activation(s[:], ys[sw][:], AF.Sqrt,
                                              bias=-1.0)
                if first and last_d_inst[0] is not None:
                    add_dep_helper(s_inst.ins, last_d_inst[0].ins,
                                   sync=False, reason="ACT phase order")
                last_s_inst[0] = s_inst
                z = zpool.tile([TILE, 2 * SW], dt.float16, tag="z")
                nc.vector.tensor_add(z[:], us[sw][:], s[:])
                zs[sw] = z
                del us[sw], ys[sw]

            def stage3(sw, first):
                n0 = sw * SW
                d_t = dpool.tile([TILE, 2 * SW], out_dtype, tag="d")
                d_inst = nc.scalar.activation(d_t[:], zs[sw][:], AF.Ln)
                if first:
                    add_dep_helper(d_inst.ins, last_s_inst[0].ins,
                                   sync=False, reason="ACT phase order")
                last_d_inst[0] = d_inst
                for ch in range(2):
                    nc.sync.dma_start(
                        outT[ch * 128:(ch + 1) * 128, n0:n0 + SW],
                        d_t[:, ch * SW:(ch + 1) * SW])
                del zs[sw]

            # group 0 stage1 runs standalone (PE warm-up covers the ramp)
            for si in range(GROUP):
                stage1(si)
            # groups g: stage1(g) interleaved across group (g-1)'s phases
            for g in range(1, n_groups + 1):
                prev = (g - 1) * GROUP
                for k in range(2 * GROUP):
                    if k % 2 == 0 and g < n_groups:
                        sw = g * GROUP + k // 2
                        if sw < NSW:
                            stage1(sw)
                    if k < GROUP:
                        stage2(prev + k, first=(k == 0))
                    else:
                        stage3(prev + (k - GROUP), first=(k == GROUP))

    nc.compile()
    return nc


def get_program(**kw):
    key = tuple(sorted(kw.items()))
    if key not in _PROGRAM_CACHE:
        _PROGRAM_CACHE[key] = build_program(**kw)
    return _PROGRAM_CACHE[key]


Y_ON_DVE = True


def _hi_lo(v):
    hi = v.astype(BF16)
    lo = (v - hi.astype(np.float32)).astype(BF16)
    return hi, lo


def host_prep(node_repr, mask, centroid_weight):
    """Build per-core xhat shards and the replicated cmov matrix."""
    x = np.ascontiguousarray(node_repr, dtype=np.float32)
    m = np.ascontiguousarray(mask, dtype=np.float32).reshape(-1)
    c = np.ascontiguousarray(centroid_weight, dtype=np.float32)

    sx = np.einsum("nd,nd->n", x, x, dtype=np.float32)
    sc = np.einsum("cd,cd->c", c, c, dtype=np.float32)
    a = m / (1.0 - sx)                      # mask folded in
    b = 2.0 / (1.0 - sc)

    # moving operand [K, C]
    cmov = np.zeros((K, C), dtype=BF16)
    cmov[0:D, :] = (-2.0 * b[:, None] * c).T.astype(BF16)
    cmov[D, :] = (b * sc).astype(BF16)
    bhi, blo = _hi_lo(b)
    cmov[D + 1, :] = bhi
    cmov[D + 2, :] = bhi
    cmov[D + 3, :] = blo
    cmov[D + 4, :] = blo

    v = a * sx
    vhi, vlo = _hi_lo(v)
    ax = (x * a[:, None]).astype(BF16)      # [N, D]
    abf = a.astype(BF16)

    xhats = []
    for i in range(N_CORES):
        n0, n1 = i * N_PER, (i + 1) * N_PER
        xh = np.zeros((K, N_PAD), dtype=BF16)
        xh[0:D, :N_PER] = ax[n0:n1].T
        xh[D, :N_PER] = abf[n0:n1]
        xh[D + 1, :N_PER] = vhi[n0:n1]
        xh[D + 2, :N_PER] = vlo[n0:n1]
        xh[D + 3, :N_PER] = vhi[n0:n1]
        xh[D + 4, :N_PER] = vlo[n0:n1]
        xhats.append(xh)
    return xhats, cmov, m


def kernel(node_repr, mask, centroid_weight, trace=False, out_dtype="float16"):
    xhats, cmov, m = host_prep(node_repr, mask, centroid_weight)

    odt = mybir.dt.float32 if out_dtype == "float32" else mybir.dt.float16
    nc = get_program(out_dtype=odt)

    in_maps = [{"xhat": xhats[i], "cmov": cmov} for i in range(N_CORES)]
    res = run_bass_kernel_spmd(nc, in_maps, core_ids=list(range(N_CORES)),
                               trace=trace)

    parts = []
    gsum = np.zeros((C,), dtype=np.float32)
    for i in range(N_CORES):
        o = np.asarray(res.results[i]["outT"][:, :N_PER], dtype=np.float32)
        gsum += o.sum(axis=1, dtype=np.float32)
        parts.append(o.T)

    node_centroid_dist = np.ascontiguousarray(
        np.concatenate(parts, axis=0))[None]  # [1, N, C]
    msum = m.sum(dtype=np.float32)
    graph_centroid_dist = (gsum / msum)[None]
    if trace:
        kernel.last_result = res
    return graph_centroid_dist, node_centroid_dist
